# revision 15
# baseline (speedup 1.0000x reference)
"""Causal attention with KV cache — Trainium2 Bass kernel, 8-core SPMD.

Sharding: batch (2) x head-group (4 heads each) = 8 cores.
Each core computes, for its (batch b, heads 4g..4g+3):
  Q/K/V projections (bf16 matmuls, fp32 psum),
  streaming softmax(QK^T)V with the KV cache (no max subtraction -- scores
  are O(5) here so exp is safe), and its partial output projection
  y_partial = attn_out @ wo[:, heads].T  (shape [T, C], bf16).
Host sums the 4 head-group partials per batch.

Device data layouts (host pre-arranges everything, bf16):
  x    [128 ci, NT ti, CO co, 512 t]  = x[b].T  split c=(co ci), t=(ti t)
  wq/wk [128 ci, HL h, CO co, HD d]   = w[rows].T split c/h (wq scaled hd^-0.5)
  wv   [128 ci, CO co, DL d]
  wo   [128 di, HL do, C c]           = wo[:, rows].T split d=(do di)
  kc   [128 d, HL h, SCO so, 128 si]
  vc   [128 si, SCO so, HL h, 128 d]
  mk   [128 si, 4 m, 512 t]           causal masks for the 4 diagonal chunks
  sel  [128 k, HL h, 128 m]           one-hot row-selector: sel[k,h,m]=(k==32h)
Output:
  y    [T, C] bf16 (partial sum over this core's heads; host sums in fp32)

Perf structure (vs the original two-phase baseline):
  - phase A is t-chunk-outer with per-head wq/wk DMAs so the first matmul
    only needs ~2.5MB of input, cutting the startup DMA stall.
  - phase B runs each t-chunk as ONE flat pair stream across all 4 heads
    (software-pipelined QK lookahead crosses head boundaries), so the PE
    never drains on the per-head softmax-denominator close.
  - softmax pair-sums run on DVE; pairs are combined into quads on GpSimd
    and the denominator ones-matmul runs per quad, except each head's last
    two pairs which feed the ones-matmul directly (no GpSimd latency on the
    close path).
  - the reciprocal broadcast is a one-hot-selector matmul into PSUM (no
    DRAM round trip), and the whole normalization + y-emission of t-chunk
    ti-1 is deferred into t-chunk ti's pair stream so it fills PE slack
    instead of head-of-line blocking it.
"""

import os
import sys

import numpy as np

for _p in ("/opt/trn_rl_repo", os.path.expanduser("~/.axon_site/_ro/trn_rl_repo")):
    if _p not in sys.path and os.path.isdir(_p):
        sys.path.insert(0, _p)

import ml_dtypes  # noqa: E402

import concourse.bass as bass  # noqa: E402
import concourse.tile as tile  # noqa: E402
from concourse import mybir  # noqa: E402

BF16 = mybir.dt.bfloat16
F32 = mybir.dt.float32
P = 128

# Full-size problem constants
B, T, C, H, HD, START = 2, 2048, 2048, 16, 128, 1024
N_CORES = 8
N_GROUPS = N_CORES // B     # 4 head groups per batch
HL = H // N_GROUPS          # 4 local heads per core
TCH = 512  # t-chunk (psum free dim)


def build_nc(T_=T, C_=C, HL_=HL, SC_=START):
    """Build the per-core Bass module. All cores run the identical program."""
    nc = bass.Bass("TRN2", target_bir_lowering=False)

    CO = C_ // P            # contraction chunks for projections
    DL = HL_ * HD           # local head dims (512)
    NT = T_ // TCH          # query t-chunks
    TSUB = TCH // P         # 128-row subchunks per t-chunk (= #diagonal masks)
    SCO = SC_ // P          # cache s-chunks
    NCC = C_ // TCH         # output column chunks

    x_d = nc.dram_tensor("x", [P, NT, CO, TCH], BF16, kind="ExternalInput")
    wq_d = nc.dram_tensor("wq", [P, HL_, CO, HD], BF16, kind="ExternalInput")
    wk_d = nc.dram_tensor("wk", [P, HL_, CO, HD], BF16, kind="ExternalInput")
    wv_d = nc.dram_tensor("wv", [P, CO, DL], BF16, kind="ExternalInput")
    wo_d = nc.dram_tensor("wo", [P, HL_, C_], BF16, kind="ExternalInput")
    kc_d = nc.dram_tensor("kc", [P, HL_, SCO, P], BF16, kind="ExternalInput")
    vc_d = nc.dram_tensor("vc", [P, SCO, HL_, P], BF16, kind="ExternalInput")
    mk_d = nc.dram_tensor("mk", [P, TSUB, TCH], BF16, kind="ExternalInput")
    sel_d = nc.dram_tensor("sel", [P, HL_, P], BF16, kind="ExternalInput")
    y_d = nc.dram_tensor("y", [T_, C_], BF16, kind="ExternalOutput")

    with tile.TileContext(nc) as tc:
        with tc.tile_pool(name="consts", bufs=1) as consts:
            # ---- persistent SBUF state (alive across both phases) ----
            ones_col = consts.tile([P, 1], BF16)
            nc.vector.memset(ones_col[:], 1.0)

            qt_sb = consts.tile([P, HL_, T_], BF16)   # Q^T  [d, h, t]
            kt_sb = consts.tile([P, HL_, T_], BF16)   # K^T new  [d, h, t]
            vn_sb = consts.tile([P, T_ // P, DL], BF16)  # V new  [t_i, t_o, d]
            kc_sb = consts.tile([P, HL_, SCO, P], BF16)
            vc_sb = consts.tile([P, SCO, HL_, P], BF16)
            mk_sb = consts.tile([P, TSUB, TCH], BF16)
            sel_sb = consts.tile([P, HL_, P], BF16)

            # ---- phase A: projections, t-chunk-outer for early start ----
            with (
                tc.tile_pool(name="projp", bufs=1) as projp,
                tc.tile_pool(name="pa_psum", bufs=1, space="PSUM") as pa_psum,
            ):
                wq_sb = projp.tile([P, HL_, CO, HD], BF16)
                wk_sb = projp.tile([P, HL_, CO, HD], BF16)
                wv_sb = projp.tile([P, CO, DL], BF16)
                xts = [projp.tile([P, CO, TCH], BF16, name=f"xt{i}")
                       for i in range(NT)]
                # DMA order: first compute group needs only wq[h0] + x[ti=0];
                # x[0] split in co-quarters so its matmul chain starts early
                nc.sync.dma_start(out=wq_sb[:, 0], in_=wq_d[:, 0])
                for q in range(4):
                    cs = slice(q * (CO // 4), (q + 1) * (CO // 4))
                    nc.sync.dma_start(out=xts[0][:, cs], in_=x_d[:, 0, cs])
                for h in range(1, HL_):
                    nc.sync.dma_start(out=wq_sb[:, h], in_=wq_d[:, h])
                for h in range(HL_):
                    nc.sync.dma_start(out=wk_sb[:, h], in_=wk_d[:, h])
                nc.sync.dma_start(out=wv_sb[:], in_=wv_d[:])
                for i in range(1, NT):
                    nc.sync.dma_start(out=xts[i][:], in_=x_d[:, i])
                nc.sync.dma_start(out=kc_sb[:], in_=kc_d[:])
                nc.sync.dma_start(out=vc_sb[:], in_=vc_d[:])
                nc.sync.dma_start(out=mk_sb[:], in_=mk_d[:])
                nc.sync.dma_start(out=sel_sb[:], in_=sel_d[:])

                for ti in range(NT):
                    xt = xts[ti]
                    # Q^T then K^T for this t-chunk: psum [d=128, t=512]
                    for wsb, dst in ((wq_sb, qt_sb), (wk_sb, kt_sb)):
                        for h in range(HL_):
                            pp = pa_psum.tile([P, TCH], F32, tag="pa", bufs=2,
                                              name="pp_qk")
                            for co in range(CO):
                                nc.tensor.matmul(
                                    pp[:],
                                    wsb[:, h, co, :],
                                    xt[:, co, :],
                                    start=(co == 0),
                                    stop=(co == CO - 1),
                                )
                            nc.scalar.copy(
                                out=dst[:, h, ti * TCH:(ti + 1) * TCH],
                                in_=pp[:],
                            )
                    # V in [t, d] layout: psum [t=128, d=DL]
                    for tt in range(TSUB):
                        tg = ti * TSUB + tt
                        pv = pa_psum.tile([P, DL], F32, tag="pa", bufs=2,
                                          name="pv")
                        for co in range(CO):
                            nc.tensor.matmul(
                                pv[:],
                                xt[:, co, tt * P:(tt + 1) * P],
                                wv_sb[:, co, :],
                                start=(co == 0),
                                stop=(co == CO - 1),
                            )
                        nc.scalar.copy(out=vn_sb[:, tg, :], in_=pv[:])

            # ---- phase B: attention + output projection ----
            with (
                tc.tile_pool(name="work", bufs=1) as work,
                tc.tile_pool(name="psum", bufs=1, space="PSUM") as psum,
            ):
                wo_sb = work.tile([P, HL_, C_], BF16, name="wo_sb")
                nc.sync.dma_start(out=wo_sb[:], in_=wo_d[:])

                onorms = {}
                osbs = {}    # (ti, h) -> unnormalized attn out in sbuf
                den4s = {}   # ti -> packed denominators (rows 0/32/64/96)

                def make_post_ops(ti, tag_cycle=("py",), evac_alt=False):
                    """Deferred normalization + y emission for t-chunk ti.
                    Returns thunks: recip, 4x (broadcast+normalize), then one
                    thunk per output column chunk of y."""
                    thunks = []
                    recb = work.tile([P, TCH], BF16, tag="recb", bufs=2,
                                     name="recb")
                    recf = work.tile([P, TCH], F32, tag="recf", bufs=2,
                                     name="recf")

                    # reciprocal in four FD=128 pieces: a monolithic [P,512]
                    # reciprocal is ~3.4us on DVE and head-of-line blocks the
                    # next chunk's pair sums (whose quads gate PE dps matmuls)
                    for q in range(4):
                        def recip_op(q=q):
                            sl = slice(q * (TCH // 4), (q + 1) * (TCH // 4))
                            nc.vector.reciprocal(out=recf[:, sl],
                                                 in_=den4s[ti][:, sl])
                        thunks.append(recip_op)

                    def recb_op():
                        nc.vector.tensor_copy(out=recb[:], in_=recf[:])
                    thunks.append(recb_op)

                    for h in range(HL_):
                        def norm_op(h=h):
                            bc = psum.tile([P, TCH], F32, tag="py", bufs=1,
                                           name="bc")
                            nc.tensor.matmul(bc[:], sel_sb[:, h, :], recb[:],
                                             start=True, stop=True)
                            nc.vector.tensor_mul(onorms[ti][:, h, :],
                                                 osbs[(ti, h)][:], bc[:])
                        thunks.append(norm_op)

                    idx = 0
                    for ci in range(NCC):
                        for tsub in range(TSUB):
                            tag = tag_cycle[idx % len(tag_cycle)]
                            on_act = evac_alt and (idx % 2 == 1)
                            idx += 1

                            def emit_op(ci=ci, tsub=tsub, tag=tag,
                                        on_act=on_act):
                                py = psum.tile([P, TCH], F32, tag=tag,
                                               bufs=(2 if tag == "acc" else 1),
                                               name="py")
                                onorm = onorms[ti]
                                for h in range(HL_):
                                    nc.tensor.matmul(
                                        py[:],
                                        onorm[:, h, tsub * P:(tsub + 1) * P],
                                        wo_sb[:, h, ci * TCH:(ci + 1) * TCH],
                                        start=(h == 0),
                                        stop=(h == HL_ - 1),
                                    )
                                ysb = work.tile([P, TCH], BF16, tag="ysb",
                                                bufs=3, name="ysb")
                                if on_act:
                                    nc.scalar.copy(out=ysb[:], in_=py[:])
                                else:
                                    nc.vector.tensor_copy(out=ysb[:],
                                                          in_=py[:])
                                t0 = ti * TCH + tsub * P
                                nc.sync.dma_start(
                                    out=y_d[t0:t0 + P, ci * TCH:(ci + 1) * TCH],
                                    in_=ysb[:],
                                )
                            thunks.append(emit_op)
                    return thunks

                # one flat software-pipelined pair stream over ALL t-chunks
                # and heads: the QK lookahead crosses head AND t-chunk
                # boundaries so the PE never drains at a close.
                def n_pair_of(ti):
                    return (SCO + (ti + 1) * TSUB) // 2

                slots = [(ti, h, p)
                         for ti in range(NT)
                         for h in range(HL_)
                         for p in range(n_pair_of(ti))]

                def kt_of(ti, h, j):
                    if j < SCO:
                        return kc_sb[:, h, j, :]
                    sn = j - SCO
                    return kt_sb[:, h, sn * P:(sn + 1) * P]

                def v_of(ti, h, j):
                    if j < SCO:
                        return vc_sb[:, j, h, :]
                    sn = j - SCO
                    return vn_sb[:, sn, h * HD:(h + 1) * HD]

                def qk_pair(slot):
                    ti, h, p = slot
                    s2 = psum.tile([P, 2, TCH], F32, tag="S2", bufs=2,
                                   name="s2")
                    q_rhs = qt_sb[:, h, ti * TCH:(ti + 1) * TCH]
                    for i in (0, 1):
                        nc.tensor.matmul(s2[:, i, :], kt_of(ti, h, 2 * p + i),
                                         q_rhs, start=True, stop=True)
                    return s2

                accs = {}
                dens = {}
                padds = {}
                quads = {}
                dps_done = {}
                post_queue = []
                post_when = []
                idx_in_ti = 0

                def dps_mm(h, src, first, last):
                    nc.tensor.matmul(dens[h][:1, :], ones_col[:], src[:],
                                     start=first, stop=last)

                s2_prev = qk_pair(slots[0])
                for idx, (ti, h, p) in enumerate(slots):
                    s2_next = (qk_pair(slots[idx + 1])
                               if idx + 1 < len(slots) else None)
                    n_s = SCO + (ti + 1) * TSUB
                    n_pair = n_s // 2
                    n_quad = (n_pair - 2) // 2
                    diag0 = n_s - TSUB

                    if h == 0 and p == 0:
                        # entering a new t-chunk: set up its state and queue
                        # the previous chunk's normalization + y emission.
                        # Drain positions: reciprocal early, broadcast+
                        # normalize after the reciprocal's ~3.7us DVE latency
                        # so their PE matmuls never wait, emits after.
                        for th in post_queue:
                            th()
                        onorms[ti] = work.tile([P, HL_, TCH], BF16,
                                               tag="onorm", bufs=2,
                                               name="onorm")
                        den4s[ti] = work.tile([P, TCH], F32, tag="den4",
                                              bufs=2, name="den4")
                        nc.vector.memset(den4s[ti][:], 1.0)
                        post_queue = make_post_ops(ti - 1) if ti > 0 else []
                        post_when = ([3, 4, 5, 6, 7, 9, 10, 11, 12]
                                     + list(range(13, 29)))
                        idx_in_ti = 0
                    if p == 0:
                        accs[h] = psum.tile([P, TCH], F32, tag="acc",
                                            bufs=2, name="acc")
                        dens[h] = psum.tile([P, TCH], F32, tag="den",
                                            bufs=1, name="den")
                        padds[h] = []
                        quads[h] = []
                        dps_done[h] = 0

                    s2 = s2_prev
                    e2 = work.tile([P, 2, TCH], BF16, tag="E", bufs=3,
                                   name="e2")
                    nc.scalar.activation(
                        out=e2[:], in_=s2[:],
                        func=mybir.ActivationFunctionType.Exp,
                    )
                    m0 = 2 * p - diag0
                    if m0 >= 0:
                        # fully-masked diagonal pair: one fused mask multiply
                        nc.vector.tensor_mul(e2[:], e2[:],
                                             mk_sb[:, m0:m0 + 2, :])
                    for i in (0, 1):
                        j = 2 * p + i
                        nc.tensor.matmul(
                            accs[h][:], v_of(ti, h, j), e2[:, i, :],
                            start=(j == 0), stop=(j == n_s - 1),
                        )
                    padd = work.tile([P, TCH], BF16, tag="padd",
                                     bufs=8, name="padd")
                    nc.vector.tensor_add(padd[:], e2[:, 0, :], e2[:, 1, :])
                    padds[h].append(padd)
                    if p % 2 == 1 and p <= n_pair - 3:
                        qd = work.tile([P, TCH], BF16, tag="quad",
                                       bufs=4, name="quad")
                        nc.gpsimd.tensor_add(out=qd[:],
                                             in0=padds[h][-2][:],
                                             in1=padds[h][-1][:])
                        quads[h].append(qd)

                    if p == n_pair - 1:
                        # close head h: flush quads, then the last two
                        # pair sums directly (skips the GpSimd latency)
                        while dps_done[h] < n_quad:
                            dps_mm(h, quads[h][dps_done[h]],
                                   dps_done[h] == 0, False)
                            dps_done[h] += 1
                        dps_mm(h, padds[h][n_pair - 2], False, False)
                        dps_mm(h, padds[h][n_pair - 1], False, True)
                        # bufs=8: osb values of chunk ti are consumed by
                        # deferred norm ops early in chunk ti+1's stream,
                        # while ti+1's own closes already write new ones
                        osb = work.tile([P, TCH], BF16, tag="osb", bufs=8,
                                        name="osb")
                        nc.vector.tensor_copy(out=osb[:], in_=accs[h][:])
                        osbs[(ti, h)] = osb
                        nc.scalar.copy(
                            out=den4s[ti][32 * h:32 * h + 1, :],
                            in_=dens[h][:1, :])
                    else:
                        # quad flush lagged by 2: GpSimd's ~1.2us service time
                        # plus its backlog at head closes never stalls the PE
                        while dps_done[h] < len(quads[h]) - 2:
                            dps_mm(h, quads[h][dps_done[h]],
                                   dps_done[h] == 0, False)
                            dps_done[h] += 1

                    if post_queue and idx_in_ti >= post_when[0]:
                        post_queue.pop(0)()
                        post_when.pop(0)
                    idx_in_ti += 1
                    s2_prev = s2_next

                for th in post_queue:
                    th()
                # final chunk's close, triple-buffered psum + alternating
                # evacuation engines so its py chains stream back-to-back
                for th in make_post_ops(NT - 1, tag_cycle=("py", "den", "acc"),
                                        evac_alt=True):
                    th()

    # walrus allows a single sync wait per hw instruction: shed matmul extras
    # onto ldweights, then split any remaining multi-waits via event sems
    bass._bass_rust.move_matmul_waits_to_ldweights(nc.m)
    bass._bass_rust.generate_event_semaphores(nc)
    return nc


def _bf16(a):
    return np.ascontiguousarray(a).astype(ml_dtypes.bfloat16)


def make_core_inputs(x, k_cache, v_cache, wq, wk, wv, wo, core,
                     T_=T, C_=C, HL_=HL, SC_=START, n_groups=None):
    """Host-side shard + relayout for one core."""
    CO = C_ // P
    DL = HL_ * HD
    NT = T_ // TCH
    TSUB = TCH // P
    SCO = SC_ // P
    if n_groups is None:
        n_groups = (k_cache.shape[1] + HL_ - 1) // HL_
    b, g = divmod(core, n_groups)
    heads = slice(HL_ * g, HL_ * (g + 1))
    rows = slice(DL * g, DL * (g + 1))
    scale = HD ** -0.5

    # x: [ci, ti, co, t]
    xd = (x[b].T.reshape(CO, P, NT, TCH).transpose(1, 2, 0, 3))
    # wq/wk: [ci, h, co, hd]
    wqd = ((wq[rows].T * scale).reshape(CO, P, HL_, HD).transpose(1, 2, 0, 3))
    wkd = (wk[rows].T.reshape(CO, P, HL_, HD).transpose(1, 2, 0, 3))
    wvd = wv[rows].T.reshape(CO, P, DL).transpose(1, 0, 2)
    wod = wo[:, rows].T.reshape(HL_, P, C_).transpose(1, 0, 2)
    kcd = k_cache[b, heads].reshape(HL_, SCO, P, P).transpose(3, 0, 1, 2)
    vcd = v_cache[b, heads].reshape(HL_, SCO, P, P).transpose(2, 1, 0, 3)
    si = np.arange(P)[:, None, None]
    mm = np.arange(TSUB)[None, :, None]
    tt = np.arange(TCH)[None, None, :]
    mkd = (tt >= si + P * mm)
    kk = np.arange(P)[:, None, None]
    hh = np.arange(HL_)[None, :, None]
    pm = np.arange(P)[None, None, :]
    seld = ((kk == 32 * hh) & (pm >= 0))

    return {
        "x": _bf16(xd), "wq": _bf16(wqd), "wk": _bf16(wkd), "wv": _bf16(wvd),
        "wo": _bf16(wod), "kc": _bf16(kcd), "vc": _bf16(vcd),
        "mk": _bf16(mkd.astype(np.float32)),
        "sel": _bf16(seld.astype(np.float32)),
    }


_NC_CACHE = None


def _get_nc():
    global _NC_CACHE
    if _NC_CACHE is None:
        _NC_CACHE = build_nc()
    return _NC_CACHE


def run_spmd(inputs, trace=False):
    """Run the 8-core SPMD kernel; returns (y_full, BassKernelResults)."""
    from concourse.bass_utils import run_bass_kernel_spmd

    x = np.asarray(inputs["x"], dtype=np.float32)
    k_cache = np.asarray(inputs["k_cache"], dtype=np.float32)
    v_cache = np.asarray(inputs["v_cache"], dtype=np.float32)
    wq = np.asarray(inputs["wq"], dtype=np.float32)
    wk = np.asarray(inputs["wk"], dtype=np.float32)
    wv = np.asarray(inputs["wv"], dtype=np.float32)
    wo = np.asarray(inputs["wo"], dtype=np.float32)
    assert int(inputs["start_pos"]) == START

    nc = _get_nc()
    in_maps = [
        make_core_inputs(x, k_cache, v_cache, wq, wk, wv, wo, core)
        for core in range(N_CORES)
    ]
    res = run_bass_kernel_spmd(
        nc, in_maps, core_ids=list(range(N_CORES)), trace=trace
    )
    n_groups = N_CORES // B
    y = np.zeros((B, T, C), dtype=np.float32)
    for core in range(N_CORES):
        b = core // n_groups
        y[b] += np.asarray(res.results[core]["y"], dtype=np.float32)
    return y, res


def kernel(**inputs):
    y, _ = run_spmd(inputs, trace=False)
    return y


# revision 16
# speedup vs baseline: 1.0108x; 1.0108x over previous
"""Causal attention with KV cache — Trainium2 Bass kernel, 8-core SPMD.

Sharding: batch (2) x head-group (4 heads each) = 8 cores.
Each core computes, for its (batch b, heads 4g..4g+3):
  Q/K/V projections (bf16 matmuls, fp32 psum),
  streaming softmax(QK^T)V with the KV cache (no max subtraction -- scores
  are O(5) here so exp is safe), and its partial output projection
  y_partial = attn_out @ wo[:, heads].T  (shape [T, C], bf16).
Host sums the 4 head-group partials per batch.

Device data layouts (host pre-arranges everything, bf16):
  x    [128 ci, NT ti, CO co, 512 t]  = x[b].T  split c=(co ci), t=(ti t)
  wq/wk [128 ci, HL h, CO co, HD d]   = w[rows].T split c/h (wq scaled hd^-0.5)
  wv   [128 ci, CO co, DL d]
  wo   [128 di, HL do, C c]           = wo[:, rows].T split d=(do di)
  kc   [128 d, HL h, SCO so, 128 si]
  vc   [128 si, SCO so, HL h, 128 d]
  mk   [128 si, 4 m, 512 t]           causal masks for the 4 diagonal chunks
  sel  [128 k, HL h, 128 m]           one-hot row-selector: sel[k,h,m]=(k==32h)
Output:
  y    [T, C] bf16 (partial sum over this core's heads; host sums in fp32)

Perf structure (vs the original two-phase baseline):
  - phase A is t-chunk-outer with per-head wq/wk DMAs so the first matmul
    only needs ~2.5MB of input, cutting the startup DMA stall.
  - phase B runs each t-chunk as ONE flat pair stream across all 4 heads
    (software-pipelined QK lookahead crosses head boundaries), so the PE
    never drains on the per-head softmax-denominator close.
  - softmax pair-sums run on DVE; pairs are combined into quads on GpSimd
    and the denominator ones-matmul runs per quad, except each head's last
    two pairs which feed the ones-matmul directly (no GpSimd latency on the
    close path).
  - the reciprocal broadcast is a one-hot-selector matmul into PSUM (no
    DRAM round trip), and the whole normalization + y-emission of t-chunk
    ti-1 is deferred into t-chunk ti's pair stream so it fills PE slack
    instead of head-of-line blocking it.
"""

import os
import sys

import numpy as np

for _p in ("/opt/trn_rl_repo", os.path.expanduser("~/.axon_site/_ro/trn_rl_repo")):
    if _p not in sys.path and os.path.isdir(_p):
        sys.path.insert(0, _p)

import ml_dtypes  # noqa: E402

import concourse.bass as bass  # noqa: E402
import concourse.tile as tile  # noqa: E402
from concourse import mybir  # noqa: E402

BF16 = mybir.dt.bfloat16
F32 = mybir.dt.float32
P = 128

# Full-size problem constants
B, T, C, H, HD, START = 2, 2048, 2048, 16, 128, 1024
N_CORES = 8
N_GROUPS = N_CORES // B     # 4 head groups per batch
HL = H // N_GROUPS          # 4 local heads per core
TCH = 512  # t-chunk (psum free dim)


def build_nc(T_=T, C_=C, HL_=HL, SC_=START):
    """Build the per-core Bass module. All cores run the identical program."""
    nc = bass.Bass("TRN2", target_bir_lowering=False)

    CO = C_ // P            # contraction chunks for projections
    DL = HL_ * HD           # local head dims (512)
    NT = T_ // TCH          # query t-chunks
    TSUB = TCH // P         # 128-row subchunks per t-chunk (= #diagonal masks)
    SCO = SC_ // P          # cache s-chunks
    NCC = C_ // TCH         # output column chunks

    x_d = nc.dram_tensor("x", [P, NT, CO, TCH], BF16, kind="ExternalInput")
    wq_d = nc.dram_tensor("wq", [P, HL_, CO, HD], BF16, kind="ExternalInput")
    wk_d = nc.dram_tensor("wk", [P, HL_, CO, HD], BF16, kind="ExternalInput")
    wv_d = nc.dram_tensor("wv", [P, CO, DL], BF16, kind="ExternalInput")
    wo_d = nc.dram_tensor("wo", [P, HL_, C_], BF16, kind="ExternalInput")
    kc_d = nc.dram_tensor("kc", [P, HL_, SCO, P], BF16, kind="ExternalInput")
    vc_d = nc.dram_tensor("vc", [P, SCO, HL_, P], BF16, kind="ExternalInput")
    mk_d = nc.dram_tensor("mk", [P, TSUB, TCH], BF16, kind="ExternalInput")
    sel_d = nc.dram_tensor("sel", [P, HL_, P], BF16, kind="ExternalInput")
    y_d = nc.dram_tensor("y", [T_, C_], BF16, kind="ExternalOutput")

    with tile.TileContext(nc) as tc:
        with tc.tile_pool(name="consts", bufs=1) as consts:
            # ---- persistent SBUF state (alive across both phases) ----
            ones_col = consts.tile([P, 1], BF16)
            nc.vector.memset(ones_col[:], 1.0)

            qt_sb = consts.tile([P, HL_, T_], BF16)   # Q^T  [d, h, t]
            kt_sb = consts.tile([P, HL_, T_], BF16)   # K^T new  [d, h, t]
            vn_sb = consts.tile([P, T_ // P, DL], BF16)  # V new  [t_i, t_o, d]
            kc_sb = consts.tile([P, HL_, SCO, P], BF16)
            vc_sb = consts.tile([P, SCO, HL_, P], BF16)
            mk_sb = consts.tile([P, TSUB, TCH], BF16)
            sel_sb = consts.tile([P, HL_, P], BF16)

            # ---- phase A: projections, t-chunk-outer for early start ----
            with (
                tc.tile_pool(name="projp", bufs=1) as projp,
                tc.tile_pool(name="pa_psum", bufs=1, space="PSUM") as pa_psum,
            ):
                wq_sb = projp.tile([P, HL_, CO, HD], BF16)
                wk_sb = projp.tile([P, HL_, CO, HD], BF16)
                wv_sb = projp.tile([P, CO, DL], BF16)
                xts = [projp.tile([P, CO, TCH], BF16, name=f"xt{i}")
                       for i in range(NT)]
                # DMA order: first compute group needs only wq[h0] + x[ti=0];
                # x[0] split in co-quarters so its matmul chain starts early
                nc.sync.dma_start(out=wq_sb[:, 0], in_=wq_d[:, 0])
                for q in range(4):
                    cs = slice(q * (CO // 4), (q + 1) * (CO // 4))
                    nc.sync.dma_start(out=xts[0][:, cs], in_=x_d[:, 0, cs])
                for h in range(1, HL_):
                    nc.sync.dma_start(out=wq_sb[:, h], in_=wq_d[:, h])
                for h in range(HL_):
                    nc.sync.dma_start(out=wk_sb[:, h], in_=wk_d[:, h])
                nc.sync.dma_start(out=wv_sb[:], in_=wv_d[:])
                for i in range(1, NT):
                    nc.sync.dma_start(out=xts[i][:], in_=x_d[:, i])
                nc.sync.dma_start(out=kc_sb[:], in_=kc_d[:])
                nc.sync.dma_start(out=vc_sb[:], in_=vc_d[:])
                nc.sync.dma_start(out=mk_sb[:], in_=mk_d[:])
                nc.sync.dma_start(out=sel_sb[:], in_=sel_d[:])

                for ti in range(NT):
                    xt = xts[ti]
                    # Q^T then K^T for this t-chunk: psum [d=128, t=512]
                    for wsb, dst in ((wq_sb, qt_sb), (wk_sb, kt_sb)):
                        for h in range(HL_):
                            pp = pa_psum.tile([P, TCH], F32, tag="pa", bufs=2,
                                              name="pp_qk")
                            for co in range(CO):
                                nc.tensor.matmul(
                                    pp[:],
                                    wsb[:, h, co, :],
                                    xt[:, co, :],
                                    start=(co == 0),
                                    stop=(co == CO - 1),
                                )
                            nc.scalar.copy(
                                out=dst[:, h, ti * TCH:(ti + 1) * TCH],
                                in_=pp[:],
                            )
                    # V in [t, d] layout: psum [t=128, d=DL]
                    for tt in range(TSUB):
                        tg = ti * TSUB + tt
                        pv = pa_psum.tile([P, DL], F32, tag="pa", bufs=2,
                                          name="pv")
                        for co in range(CO):
                            nc.tensor.matmul(
                                pv[:],
                                xt[:, co, tt * P:(tt + 1) * P],
                                wv_sb[:, co, :],
                                start=(co == 0),
                                stop=(co == CO - 1),
                            )
                        nc.scalar.copy(out=vn_sb[:, tg, :], in_=pv[:])

            # ---- phase B: attention + output projection ----
            with (
                tc.tile_pool(name="work", bufs=1) as work,
                tc.tile_pool(name="psum", bufs=1, space="PSUM") as psum,
            ):
                wo_sb = work.tile([P, HL_, C_], BF16, name="wo_sb")
                nc.sync.dma_start(out=wo_sb[:], in_=wo_d[:])

                onorms = {}
                osbs = {}    # (ti, h) -> unnormalized attn out in sbuf
                den4s = {}   # ti -> packed denominators (rows 0/32/64/96)

                def make_post_ops(ti, tag_cycle=("py",), evac_alt=False):
                    """Deferred normalization + y emission for t-chunk ti.
                    Returns thunks: recip, 4x (broadcast+normalize), then one
                    thunk per output column chunk of y."""
                    thunks = []
                    recb = work.tile([P, TCH], BF16, tag="recb", bufs=2,
                                     name="recb")
                    recf = work.tile([P, TCH], F32, tag="recf", bufs=2,
                                     name="recf")

                    # reciprocal in four FD=128 pieces: a monolithic [P,512]
                    # reciprocal is ~3.4us on DVE and head-of-line blocks the
                    # next chunk's pair sums (whose quads gate PE dps matmuls)
                    for q in range(4):
                        def recip_op(q=q):
                            sl = slice(q * (TCH // 4), (q + 1) * (TCH // 4))
                            nc.vector.reciprocal(out=recf[:, sl],
                                                 in_=den4s[ti][:, sl])
                        thunks.append(recip_op)

                    def recb_op():
                        nc.vector.tensor_copy(out=recb[:], in_=recf[:])
                    thunks.append(recb_op)

                    for h in range(HL_):
                        def norm_op(h=h):
                            bc = psum.tile([P, TCH], F32, tag="py", bufs=1,
                                           name="bc")
                            nc.tensor.matmul(bc[:], sel_sb[:, h, :], recb[:],
                                             start=True, stop=True)
                            nc.vector.tensor_mul(onorms[ti][:, h, :],
                                                 osbs[(ti, h)][:], bc[:])
                        thunks.append(norm_op)

                    idx = 0
                    for ci in range(NCC):
                        for tsub in range(TSUB):
                            tag = tag_cycle[idx % len(tag_cycle)]
                            on_act = evac_alt and (idx % 2 == 1)
                            idx += 1

                            def emit_op(ci=ci, tsub=tsub, tag=tag,
                                        on_act=on_act):
                                py = psum.tile([P, TCH], F32, tag=tag,
                                               bufs=(2 if tag == "acc" else 1),
                                               name="py")
                                onorm = onorms[ti]
                                for h in range(HL_):
                                    nc.tensor.matmul(
                                        py[:],
                                        onorm[:, h, tsub * P:(tsub + 1) * P],
                                        wo_sb[:, h, ci * TCH:(ci + 1) * TCH],
                                        start=(h == 0),
                                        stop=(h == HL_ - 1),
                                    )
                                ysb = work.tile([P, TCH], BF16, tag="ysb",
                                                bufs=3, name="ysb")
                                if on_act:
                                    nc.scalar.copy(out=ysb[:], in_=py[:])
                                else:
                                    nc.vector.tensor_copy(out=ysb[:],
                                                          in_=py[:])
                                t0 = ti * TCH + tsub * P
                                nc.sync.dma_start(
                                    out=y_d[t0:t0 + P, ci * TCH:(ci + 1) * TCH],
                                    in_=ysb[:],
                                )
                            thunks.append(emit_op)
                    return thunks

                # one flat software-pipelined pair stream over ALL t-chunks
                # and heads: the QK lookahead crosses head AND t-chunk
                # boundaries so the PE never drains at a close.
                def n_pair_of(ti):
                    return (SCO + (ti + 1) * TSUB) // 2

                slots = [(ti, h, p)
                         for ti in range(NT)
                         for h in range(HL_)
                         for p in range(n_pair_of(ti))]

                def kt_of(ti, h, j):
                    if j < SCO:
                        return kc_sb[:, h, j, :]
                    sn = j - SCO
                    return kt_sb[:, h, sn * P:(sn + 1) * P]

                def v_of(ti, h, j):
                    if j < SCO:
                        return vc_sb[:, j, h, :]
                    sn = j - SCO
                    return vn_sb[:, sn, h * HD:(h + 1) * HD]

                def qk_pair(slot):
                    ti, h, p = slot
                    s2 = psum.tile([P, 2, TCH], F32, tag="S2", bufs=2,
                                   name="s2")
                    q_rhs = qt_sb[:, h, ti * TCH:(ti + 1) * TCH]
                    for i in (0, 1):
                        nc.tensor.matmul(s2[:, i, :], kt_of(ti, h, 2 * p + i),
                                         q_rhs, start=True, stop=True)
                    return s2

                accs = {}
                dens = {}
                padds = {}
                quads = {}
                dps_done = {}
                post_queue = []
                post_when = []
                idx_in_ti = 0

                def dps_mm(h, src, first, last):
                    nc.tensor.matmul(dens[h][:1, :], ones_col[:], src[:],
                                     start=first, stop=last)

                s2_prev = qk_pair(slots[0])
                for idx, (ti, h, p) in enumerate(slots):
                    s2_next = (qk_pair(slots[idx + 1])
                               if idx + 1 < len(slots) else None)
                    n_s = SCO + (ti + 1) * TSUB
                    n_pair = n_s // 2
                    n_quad = (n_pair - 2) // 2
                    diag0 = n_s - TSUB

                    if h == 0 and p == 0:
                        # entering a new t-chunk: set up its state and queue
                        # the previous chunk's normalization + y emission.
                        # Drain positions: reciprocal early, broadcast+
                        # normalize after the reciprocal's ~3.7us DVE latency
                        # so their PE matmuls never wait, emits after.
                        for th in post_queue:
                            th()
                        onorms[ti] = work.tile([P, HL_, TCH], BF16,
                                               tag="onorm", bufs=2,
                                               name="onorm")
                        den4s[ti] = work.tile([P, TCH], F32, tag="den4",
                                              bufs=2, name="den4")
                        nc.vector.memset(den4s[ti][:], 1.0)
                        post_queue = make_post_ops(ti - 1) if ti > 0 else []
                        post_when = ([3, 4, 5, 6, 7, 10, 11, 12, 13]
                                     + list(range(14, 30)))
                        idx_in_ti = 0
                    if p == 0:
                        accs[h] = psum.tile([P, TCH], F32, tag="acc",
                                            bufs=2, name="acc")
                        dens[h] = psum.tile([P, TCH], F32, tag="den",
                                            bufs=1, name="den")
                        padds[h] = []
                        quads[h] = []
                        dps_done[h] = 0

                    s2 = s2_prev
                    e2 = work.tile([P, 2, TCH], BF16, tag="E", bufs=5,
                                   name="e2")
                    nc.scalar.activation(
                        out=e2[:], in_=s2[:],
                        func=mybir.ActivationFunctionType.Exp,
                    )
                    m0 = 2 * p - diag0
                    if m0 >= 0:
                        # fully-masked diagonal pair: one fused mask multiply
                        nc.vector.tensor_mul(e2[:], e2[:],
                                             mk_sb[:, m0:m0 + 2, :])
                    for i in (0, 1):
                        j = 2 * p + i
                        nc.tensor.matmul(
                            accs[h][:], v_of(ti, h, j), e2[:, i, :],
                            start=(j == 0), stop=(j == n_s - 1),
                        )
                    padd = work.tile([P, TCH], BF16, tag="padd",
                                     bufs=8, name="padd")
                    nc.vector.tensor_add(padd[:], e2[:, 0, :], e2[:, 1, :])
                    padds[h].append(padd)
                    if p % 2 == 1 and p <= n_pair - 3:
                        qd = work.tile([P, TCH], BF16, tag="quad",
                                       bufs=4, name="quad")
                        nc.gpsimd.tensor_add(out=qd[:],
                                             in0=padds[h][-2][:],
                                             in1=padds[h][-1][:])
                        quads[h].append(qd)

                    if p == n_pair - 1:
                        # close head h: flush quads, then the last two
                        # pair sums directly (skips the GpSimd latency)
                        while dps_done[h] < n_quad:
                            dps_mm(h, quads[h][dps_done[h]],
                                   dps_done[h] == 0, False)
                            dps_done[h] += 1
                        dps_mm(h, padds[h][n_pair - 2], False, False)
                        dps_mm(h, padds[h][n_pair - 1], False, True)
                        # bufs=8: osb values of chunk ti are consumed by
                        # deferred norm ops early in chunk ti+1's stream,
                        # while ti+1's own closes already write new ones
                        osb = work.tile([P, TCH], BF16, tag="osb", bufs=8,
                                        name="osb")
                        nc.vector.tensor_copy(out=osb[:], in_=accs[h][:])
                        osbs[(ti, h)] = osb
                        nc.scalar.copy(
                            out=den4s[ti][32 * h:32 * h + 1, :],
                            in_=dens[h][:1, :])
                    else:
                        # quad flush lagged by 2: GpSimd's ~1.2us service time
                        # plus its backlog at head closes never stalls the PE
                        while dps_done[h] < len(quads[h]) - 3:
                            dps_mm(h, quads[h][dps_done[h]],
                                   dps_done[h] == 0, False)
                            dps_done[h] += 1

                    if post_queue and idx_in_ti >= post_when[0]:
                        post_queue.pop(0)()
                        post_when.pop(0)
                    idx_in_ti += 1
                    s2_prev = s2_next

                for th in post_queue:
                    th()
                # final chunk's close, triple-buffered psum + alternating
                # evacuation engines so its py chains stream back-to-back
                for th in make_post_ops(NT - 1, tag_cycle=("py", "den", "acc"),
                                        evac_alt=True):
                    th()

    # walrus allows a single sync wait per hw instruction: shed matmul extras
    # onto ldweights, then split any remaining multi-waits via event sems
    bass._bass_rust.move_matmul_waits_to_ldweights(nc.m)
    bass._bass_rust.generate_event_semaphores(nc)
    return nc


def _bf16(a):
    return np.ascontiguousarray(a).astype(ml_dtypes.bfloat16)


def make_core_inputs(x, k_cache, v_cache, wq, wk, wv, wo, core,
                     T_=T, C_=C, HL_=HL, SC_=START, n_groups=None):
    """Host-side shard + relayout for one core."""
    CO = C_ // P
    DL = HL_ * HD
    NT = T_ // TCH
    TSUB = TCH // P
    SCO = SC_ // P
    if n_groups is None:
        n_groups = (k_cache.shape[1] + HL_ - 1) // HL_
    b, g = divmod(core, n_groups)
    heads = slice(HL_ * g, HL_ * (g + 1))
    rows = slice(DL * g, DL * (g + 1))
    scale = HD ** -0.5

    # x: [ci, ti, co, t]
    xd = (x[b].T.reshape(CO, P, NT, TCH).transpose(1, 2, 0, 3))
    # wq/wk: [ci, h, co, hd]
    wqd = ((wq[rows].T * scale).reshape(CO, P, HL_, HD).transpose(1, 2, 0, 3))
    wkd = (wk[rows].T.reshape(CO, P, HL_, HD).transpose(1, 2, 0, 3))
    wvd = wv[rows].T.reshape(CO, P, DL).transpose(1, 0, 2)
    wod = wo[:, rows].T.reshape(HL_, P, C_).transpose(1, 0, 2)
    kcd = k_cache[b, heads].reshape(HL_, SCO, P, P).transpose(3, 0, 1, 2)
    vcd = v_cache[b, heads].reshape(HL_, SCO, P, P).transpose(2, 1, 0, 3)
    si = np.arange(P)[:, None, None]
    mm = np.arange(TSUB)[None, :, None]
    tt = np.arange(TCH)[None, None, :]
    mkd = (tt >= si + P * mm)
    kk = np.arange(P)[:, None, None]
    hh = np.arange(HL_)[None, :, None]
    pm = np.arange(P)[None, None, :]
    seld = ((kk == 32 * hh) & (pm >= 0))

    return {
        "x": _bf16(xd), "wq": _bf16(wqd), "wk": _bf16(wkd), "wv": _bf16(wvd),
        "wo": _bf16(wod), "kc": _bf16(kcd), "vc": _bf16(vcd),
        "mk": _bf16(mkd.astype(np.float32)),
        "sel": _bf16(seld.astype(np.float32)),
    }


_NC_CACHE = None


def _get_nc():
    global _NC_CACHE
    if _NC_CACHE is None:
        _NC_CACHE = build_nc()
    return _NC_CACHE


def run_spmd(inputs, trace=False):
    """Run the 8-core SPMD kernel; returns (y_full, BassKernelResults)."""
    from concourse.bass_utils import run_bass_kernel_spmd

    x = np.asarray(inputs["x"], dtype=np.float32)
    k_cache = np.asarray(inputs["k_cache"], dtype=np.float32)
    v_cache = np.asarray(inputs["v_cache"], dtype=np.float32)
    wq = np.asarray(inputs["wq"], dtype=np.float32)
    wk = np.asarray(inputs["wk"], dtype=np.float32)
    wv = np.asarray(inputs["wv"], dtype=np.float32)
    wo = np.asarray(inputs["wo"], dtype=np.float32)
    assert int(inputs["start_pos"]) == START

    nc = _get_nc()
    in_maps = [
        make_core_inputs(x, k_cache, v_cache, wq, wk, wv, wo, core)
        for core in range(N_CORES)
    ]
    res = run_bass_kernel_spmd(
        nc, in_maps, core_ids=list(range(N_CORES)), trace=trace
    )
    n_groups = N_CORES // B
    y = np.zeros((B, T, C), dtype=np.float32)
    for core in range(N_CORES):
        b = core // n_groups
        y[b] += np.asarray(res.results[core]["y"], dtype=np.float32)
    return y, res


def kernel(**inputs):
    y, _ = run_spmd(inputs, trace=False)
    return y
